# revision 16
# baseline (speedup 1.0000x reference)
"""Trainium2 Bass kernel for nn_DCGN_5239860101881.

Math background (verified against the reference numerically):
  - The DCGN's "adjacency" matrix is diagonal with diag == 1.0 in fp32
    (cos(v,v) path), so einsum('xyz,abc->xbc') makes every propagate output
      out[b] = S * (sum_batch(node_conv(x)) @ W) + bias      (S = 360 / 120)
    and the reference output consists of 64 bit-identical [40,10] blocks.
  - The only computation touching the big x tensor is x.sum(axis=0).

Distribution: shard the node axis (1080 = 8 * 135) across the 8 cores.
Each core streams its [64, 135, 512] slice from HBM (DMA-bound), reduces
over batch, then runs the tiny replicated chain:
  node_conv -> @prop1_W -> gelu(360*. + b1) -> node_conv2(64x folded into w)
  -> @prop2_W -> gelu(120*. + b2) -> classifier
producing 5 of the 40 distinct output rows. No collectives needed.

Implementation notes:
  - Stream tiles are node-major [128 nodes, 512 f] per batch; the batch
    reduction runs on the otherwise-idle TensorE as accumulating PE
    transposes (psum += tile_chunk^T), which also leaves X^T in the
    f-on-partitions layout the tail matmuls need and keeps the PE HAM-warm.
  - Feature chunk 3 and the 7 leftover nodes (128..134) are reduced on DVE;
    leftovers stream in early and their reshuffle/transposes hide under
    phase A entirely.
"""

import numpy as np

B, N, F = 64, 1080, 512
H1, H2, NCLS = 784, 28, 10
P = 3
NCORES = 8
SLICE_N = N // NCORES            # 135 nodes per core
NW = SLICE_N // P                # 45 layer-1 windows per core
S2 = NW // P                     # 15 layer-2 windows per core
CR = S2 // P                     # 5 classifier rows per core
SLICE_ELEMS = SLICE_N * F        # 69120
MAIN_ELEMS = 128 * F             # 65536 (nodes 0..127)
LEFT_ELEMS = SLICE_ELEMS - MAIN_ELEMS  # 3584 (nodes 128..134)
GB = 8                           # batches per DMA group
NGROUPS = B // GB

_CACHE = {}


def _build_bass():
    import concourse.mybir as mybir
    from concourse import bacc
    from concourse.tile import TileContext

    fp32 = mybir.dt.float32
    nc = bacc.Bacc("TRN2", target_bir_lowering=False, debug=False,
                   num_devices=NCORES)

    xs = nc.dram_tensor("xs", [B, SLICE_ELEMS], fp32, kind="ExternalInput")
    w1pat = nc.dram_tensor("w1pat", [128, F], fp32, kind="ExternalInput")
    w1patl = nc.dram_tensor("w1patl", [7, F], fp32, kind="ExternalInput")
    sel = nc.dram_tensor("sel", [128, NW], fp32, kind="ExternalInput")
    sel2 = nc.dram_tensor("sel2", [7, NW], fp32, kind="ExternalInput")
    eye128 = nc.dram_tensor("eye128", [128, 128], fp32, kind="ExternalInput")
    p1wr = nc.dram_tensor("p1wr", [128, 4, H1], fp32, kind="ExternalInput")
    b1r = nc.dram_tensor("b1r", [112, 7], fp32, kind="ExternalInput")
    nc2wr = nc.dram_tensor("nc2wr", [112, 7, P], fp32, kind="ExternalInput")
    p2wr = nc.dram_tensor("p2wr", [112, 7, H2], fp32, kind="ExternalInput")
    b2c = nc.dram_tensor("b2c", [H2, 1], fp32, kind="ExternalInput")
    cw1 = nc.dram_tensor("cw1", [H2, P, 32], fp32, kind="ExternalInput")
    cb1c = nc.dram_tensor("cb1c", [32, 1], fp32, kind="ExternalInput")
    cw2 = nc.dram_tensor("cw2", [32, NCLS], fp32, kind="ExternalInput")
    cb2c = nc.dram_tensor("cb2c", [NCLS, 1], fp32, kind="ExternalInput")

    out = nc.dram_tensor("out", [NCLS, CR], fp32, kind="ExternalOutput")

    Gelu = mybir.ActivationFunctionType.Gelu
    Ident = mybir.ActivationFunctionType.Identity

    with TileContext(nc) as tc:
        with (
            tc.tile_pool(name="w", bufs=1) as wpool,
            tc.tile_pool(name="stream", bufs=3) as spool,
            tc.tile_pool(name="left", bufs=1) as lpool,
            tc.tile_pool(name="acc", bufs=1) as apool,
            tc.tile_pool(name="tail", bufs=1) as tpool,
            tc.tile_pool(name="psA", bufs=1, space="PSUM") as psA,
            tc.tile_pool(name="psB", bufs=2, space="PSUM") as psB,
            tc.tile_pool(name="dram", bufs=1, space="DRAM") as dpool,
        ):
            # identity needed by the very first transposes
            eye_sb = wpool.tile([128, 128], fp32)
            nc.scalar.dma_start(out=eye_sb, in_=eye128.ap())

            # leftover node stream (nodes 128..134), all batches, early
            llt = lpool.tile([128, B, 28], fp32)
            for q in range(4):
                src = xs.ap()[q * 16:(q + 1) * 16, MAIN_ELEMS:].rearrange(
                    "b (p f) -> p b f", p=128)
                nc.scalar.dma_start(out=llt[:, q * 16:(q + 1) * 16, :], in_=src)

            # main group DMAs issued up-front in program order.
            # Batch reduction on TensorE: psum += I.T @ tile_b, identity
            # stationary loaded once; X_bar lands node-major in one bank.
            px = psA.tile([128, F], fp32)        # one psum bank (2KB)
            for g in range(NGROUPS):
                gtm = spool.tile([128, GB, F], fp32, tag="grp")
                src = xs.ap()[g * GB:(g + 1) * GB, 0:MAIN_ELEMS].rearrange(
                    "b (n f) -> n b f", n=128)
                nc.sync.dma_start(out=gtm, in_=src)
                for b in range(GB):
                    bg = g * GB + b
                    nc.tensor.matmul(px, eye_sb, gtm[:, b, :],
                                     start=(bg == 0), stop=(bg == B - 1))

            # ---- weights (scheduled around the stream) ----
            w1pat_sb = wpool.tile([128, F], fp32)
            nc.scalar.dma_start(out=w1pat_sb, in_=w1pat.ap())
            w1patl_sb = wpool.tile([7, F], fp32)
            nc.scalar.dma_start(out=w1patl_sb, in_=w1patl.ap())
            sel_sb = wpool.tile([128, NW], fp32)
            nc.scalar.dma_start(out=sel_sb, in_=sel.ap())
            sel2_sb = wpool.tile([7, NW], fp32)
            nc.scalar.dma_start(out=sel2_sb, in_=sel2.ap())
            p1w_sb = wpool.tile([128, 4, H1], fp32)
            nc.scalar.dma_start(out=p1w_sb, in_=p1wr.ap())
            b1_sb = wpool.tile([112, 7], fp32)
            nc.scalar.dma_start(out=b1_sb, in_=b1r.ap())
            nc2w_sb = wpool.tile([112, 7, P], fp32)
            nc.scalar.dma_start(out=nc2w_sb, in_=nc2wr.ap())
            p2w_sb = wpool.tile([112, 7, H2], fp32)
            nc.scalar.dma_start(out=p2w_sb, in_=p2wr.ap())
            b2_sb = wpool.tile([H2, 1], fp32)
            nc.scalar.dma_start(out=b2_sb, in_=b2c.ap())
            cw1_sb = wpool.tile([H2, P, 32], fp32)
            nc.scalar.dma_start(out=cw1_sb, in_=cw1.ap())
            cb1_sb = wpool.tile([32, 1], fp32)
            nc.scalar.dma_start(out=cb1_sb, in_=cb1c.ap())
            cw2_sb = wpool.tile([32, NCLS], fp32)
            nc.scalar.dma_start(out=cw2_sb, in_=cw2.ap())
            cb2_sb = wpool.tile([NCLS, 1], fp32)
            nc.scalar.dma_start(out=cb2_sb, in_=cb2c.ap())

            # preload the gelu ACT table during phase A
            gdummy = tpool.tile([H2, 1], fp32)
            nc.scalar.activation(out=gdummy, in_=b2_sb, func=Gelu)

            # leftover reduction: 63 adds of [128, 28] + roundtrip
            # (both hidden under phase A)
            accl = apool.tile([128, 28], fp32)
            for b in range(B):
                if b == 0:
                    nc.vector.tensor_copy(out=accl, in_=llt[:, 0, :])
                else:
                    nc.vector.tensor_add(out=accl, in0=accl, in1=llt[:, b, :])
            scratch = dpool.tile([LEFT_ELEMS], fp32)
            nc.sync.dma_start(
                out=scratch.rearrange("(p f) -> p f", p=128), in_=accl)
            lt7 = lpool.tile([7, F], fp32)
            nc.sync.dma_start(
                out=lt7, in_=scratch.rearrange("(n f) -> n f", n=7))
            yl = lpool.tile([7, F], fp32)
            nc.vector.tensor_mul(out=yl, in0=lt7, in1=w1patl_sb)

            # ---- after the stream: drain X_bar, apply window weights ----
            xbar = tpool.tile([128, F], fp32)
            nc.vector.tensor_copy(out=xbar, in_=px)
            ymain = tpool.tile([128, F], fp32)
            nc.vector.tensor_mul(out=ymain, in0=xbar, in1=w1pat_sb)

            # hsumT[f, s] = sum_{n in window s} Y[n, f]   (Y^T @ Sel on PE)
            hsumT = tpool.tile([128, 4, NW], fp32)
            for fc in range(4):
                ph = psB.tile([128, NW], fp32, tag="ph")
                nc.tensor.matmul(ph, ymain[:, fc * 128:(fc + 1) * 128],
                                 sel_sb, start=True, stop=False)
                nc.tensor.matmul(ph, yl[:, fc * 128:(fc + 1) * 128],
                                 sel2_sb, start=False, stop=True)
                nc.vector.tensor_copy(out=hsumT[:, fc, :], in_=ph)

            # ---- M1^T chunks + gelu -> h1cT [112, 7, 45] ----
            h1cT = tpool.tile([112, 7, NW], fp32)
            for hc in range(7):
                pm = psB.tile([112, NW], fp32, tag="pm")
                for fc in range(4):
                    lhsT = p1w_sb[:, fc, hc * 112:(hc + 1) * 112]
                    nc.tensor.matmul(pm, lhsT, hsumT[:, fc, :],
                                     start=(fc == 0), stop=(fc == 3))
                nc.scalar.activation(out=h1cT[:, hc, :], in_=pm, func=Gelu,
                                     bias=b1_sb[:, hc:hc + 1], scale=360.0)

            # ---- node conv 2 (64x batch factor folded into nc2w host-side) ----
            tmp2 = tpool.tile([112, 7, NW], fp32)
            h1v = h1cT.rearrange("p c (s q) -> p c s q", q=P)
            w2v = nc2w_sb[:, :, None, :].to_broadcast((112, 7, S2, P))
            nc.vector.tensor_mul(
                out=tmp2.rearrange("p c (s q) -> p c s q", q=P),
                in0=h1v, in1=w2v)
            hs2T = tpool.tile([112, 7, S2], fp32)
            nc.vector.reduce_sum(
                out=hs2T, in_=tmp2.rearrange("p c (s q) -> p c s q", q=P),
                axis=mybir.AxisListType.X)

            # ---- M2^T [28, 15] + gelu ----
            pm2 = psB.tile([H2, S2], fp32, tag="pm")
            for c in range(7):
                nc.tensor.matmul(pm2, p2w_sb[:, c, :], hs2T[:, c, :],
                                 start=(c == 0), stop=(c == 6))
            out2T = tpool.tile([H2, S2], fp32)
            nc.scalar.activation(out=out2T, in_=pm2, func=Gelu,
                                 bias=b2_sb[:, 0:1], scale=120.0)

            # ---- classifier ----
            o2v = out2T.rearrange("h (r q) -> h r q", q=P)
            pc1 = psB.tile([32, CR], fp32, tag="pm")
            for q in range(P):
                nc.tensor.matmul(pc1, cw1_sb[:, q, :], o2v[:, :, q],
                                 start=(q == 0), stop=(q == P - 1))
            c1T = tpool.tile([32, CR], fp32)
            nc.scalar.activation(out=c1T, in_=pc1, func=Gelu,
                                 bias=cb1_sb[:, 0:1], scale=1.0)
            pc2 = psB.tile([NCLS, CR], fp32, tag="pm")
            nc.tensor.matmul(pc2, cw2_sb, c1T, start=True, stop=True)
            outT = tpool.tile([NCLS, CR], fp32)
            nc.scalar.activation(out=outT, in_=pc2, func=Ident,
                                 bias=cb2_sb[:, 0:1], scale=1.0)
            nc.sync.dma_start(out=out.ap(), in_=outT)

    nc.compile()
    return nc


def _prep_in_maps(inputs):
    x = np.ascontiguousarray(np.asarray(inputs["x"], dtype=np.float32))
    nc1_w = np.asarray(inputs["nc1_w"], dtype=np.float32)
    prop1_W = np.asarray(inputs["prop1_W"], dtype=np.float32)
    prop1_b = np.asarray(inputs["prop1_b"], dtype=np.float32)
    nc2_w = np.asarray(inputs["nc2_w"], dtype=np.float32)
    prop2_W = np.asarray(inputs["prop2_W"], dtype=np.float32)
    prop2_b = np.asarray(inputs["prop2_b"], dtype=np.float32)
    cls_w1 = np.asarray(inputs["cls_w1"], dtype=np.float32)
    cls_b1 = np.asarray(inputs["cls_b1"], dtype=np.float32)
    cls_w2 = np.asarray(inputs["cls_w2"], dtype=np.float32)
    cls_b2 = np.asarray(inputs["cls_b2"], dtype=np.float32)

    common = {
        "w1pat": np.ascontiguousarray(
            nc1_w[np.arange(128) % P, :]),
        "w1patl": np.ascontiguousarray(
            nc1_w[(128 + np.arange(7)) % P, :]),
        "sel": np.ascontiguousarray(
            (np.arange(128)[:, None] // P == np.arange(NW)[None, :])
            .astype(np.float32)),
        "sel2": np.ascontiguousarray(
            ((128 + np.arange(7))[:, None] // P == np.arange(NW)[None, :])
            .astype(np.float32)),
        "eye128": np.eye(128, dtype=np.float32),
        "p1wr": np.ascontiguousarray(
            prop1_W.reshape(4, 128, H1).swapaxes(0, 1)),
        "b1r": np.ascontiguousarray(prop1_b.reshape(7, 112).T),
        "nc2wr": np.ascontiguousarray(
            (64.0 * nc2_w).astype(np.float32).T.reshape(7, 112, P)
            .swapaxes(0, 1)),
        "p2wr": np.ascontiguousarray(prop2_W.reshape(7, 112, H2)
                                     .swapaxes(0, 1)),
        "b2c": np.ascontiguousarray(prop2_b.reshape(H2, 1)),
        "cw1": np.ascontiguousarray(cls_w1.reshape(P, H2, 32).swapaxes(0, 1)),
        "cb1c": np.ascontiguousarray(cls_b1.reshape(32, 1)),
        "cw2": np.ascontiguousarray(cls_w2),
        "cb2c": np.ascontiguousarray(cls_b2.reshape(NCLS, 1)),
    }
    in_maps = []
    for c in range(NCORES):
        xsl = np.ascontiguousarray(
            x[:, c * SLICE_N:(c + 1) * SLICE_N, :].reshape(B, SLICE_ELEMS))
        in_maps.append({"xs": xsl, **common})
    return in_maps


def run(inputs, trace=False):
    from concourse import bass_utils
    if "nc" not in _CACHE:
        _CACHE["nc"] = _build_bass()
    nc = _CACHE["nc"]
    in_maps = _prep_in_maps(inputs)
    res = bass_utils.run_bass_kernel_spmd(
        nc, in_maps, core_ids=list(range(NCORES)), trace=trace)
    outs = [np.asarray(res.results[c]["out"]) for c in range(NCORES)]
    block = np.concatenate([o.T for o in outs], axis=0)       # [40, 10]
    full = np.tile(block, (B, 1)).astype(np.float32)          # [2560, 10]
    return full, res


def kernel(**inputs) -> np.ndarray:
    out, _ = run(inputs, trace=False)
    return out


# revision 17
# speedup vs baseline: 1.1631x; 1.1631x over previous
"""Trainium2 Bass kernel for nn_DCGN_5239860101881.

Math background (verified against the reference numerically):
  - The DCGN's "adjacency" matrix is diagonal with diag == 1.0 in fp32
    (cos(v,v) path), so einsum('xyz,abc->xbc') makes every propagate output
      out[b] = S * (sum_batch(node_conv(x)) @ W) + bias      (S = 360 / 120)
    and the reference output consists of 64 bit-identical [40,10] blocks.
  - The only computation touching the big x tensor is x.sum(axis=0).

Distribution: shard the node axis (1080 = 8 * 135) across the 8 cores.
Each core streams its [64, 135, 512] slice from HBM (DMA-bound), reduces
over batch, then runs the tiny replicated chain:
  node_conv -> @prop1_W -> gelu(360*. + b1) -> node_conv2(64x folded into w)
  -> @prop2_W -> gelu(120*. + b2) -> classifier
producing 5 of the 40 distinct output rows. No collectives needed.

Implementation notes:
  - Stream tiles are node-major [128 nodes, 512 f] per batch; the batch
    reduction runs on the otherwise-idle TensorE as accumulating PE
    transposes (psum += tile_chunk^T), which also leaves X^T in the
    f-on-partitions layout the tail matmuls need and keeps the PE HAM-warm.
  - Feature chunk 3 and the 7 leftover nodes (128..134) are reduced on DVE;
    leftovers stream in early and their reshuffle/transposes hide under
    phase A entirely.
"""

import numpy as np

B, N, F = 64, 1080, 512
H1, H2, NCLS = 784, 28, 10
P = 3
NCORES = 8
SLICE_N = N // NCORES            # 135 nodes per core
NW = SLICE_N // P                # 45 layer-1 windows per core
S2 = NW // P                     # 15 layer-2 windows per core
CR = S2 // P                     # 5 classifier rows per core
SLICE_ELEMS = SLICE_N * F        # 69120
MAIN_ELEMS = 128 * F             # 65536 (nodes 0..127)
LEFT_ELEMS = SLICE_ELEMS - MAIN_ELEMS  # 3584 (nodes 128..134)
GB = 8                           # batches per DMA group
NGROUPS = B // GB

_CACHE = {}


def _build_bass():
    import concourse.mybir as mybir
    from concourse import bacc
    from concourse.tile import TileContext

    fp32 = mybir.dt.float32
    nc = bacc.Bacc("TRN2", target_bir_lowering=False, debug=False,
                   num_devices=NCORES)

    xsm = nc.dram_tensor("xsm", [B, MAIN_ELEMS], fp32, kind="ExternalInput")
    xsl = nc.dram_tensor("xsl", [128, B * 28], fp32, kind="ExternalInput")
    w1pat = nc.dram_tensor("w1pat", [128, F], fp32, kind="ExternalInput")
    w1patl = nc.dram_tensor("w1patl", [7, F], fp32, kind="ExternalInput")
    sel = nc.dram_tensor("sel", [128, NW], fp32, kind="ExternalInput")
    sel2 = nc.dram_tensor("sel2", [7, NW], fp32, kind="ExternalInput")
    eye128 = nc.dram_tensor("eye128", [128, 128], fp32, kind="ExternalInput")
    p1wr = nc.dram_tensor("p1wr", [128, 4, H1], fp32, kind="ExternalInput")
    b1r = nc.dram_tensor("b1r", [112, 7], fp32, kind="ExternalInput")
    nc2wr = nc.dram_tensor("nc2wr", [112, 7, P], fp32, kind="ExternalInput")
    p2wr = nc.dram_tensor("p2wr", [112, 7, H2], fp32, kind="ExternalInput")
    b2c = nc.dram_tensor("b2c", [H2, 1], fp32, kind="ExternalInput")
    cw1 = nc.dram_tensor("cw1", [H2, P, 32], fp32, kind="ExternalInput")
    cb1c = nc.dram_tensor("cb1c", [32, 1], fp32, kind="ExternalInput")
    cw2 = nc.dram_tensor("cw2", [32, NCLS], fp32, kind="ExternalInput")
    cb2c = nc.dram_tensor("cb2c", [NCLS, 1], fp32, kind="ExternalInput")

    out = nc.dram_tensor("out", [NCLS, CR], fp32, kind="ExternalOutput")

    Gelu = mybir.ActivationFunctionType.Gelu
    Ident = mybir.ActivationFunctionType.Identity

    with TileContext(nc) as tc:
        with (
            tc.tile_pool(name="w", bufs=1) as wpool,
            tc.tile_pool(name="stream", bufs=3) as spool,
            tc.tile_pool(name="left", bufs=1) as lpool,
            tc.tile_pool(name="acc", bufs=1) as apool,
            tc.tile_pool(name="tail", bufs=1) as tpool,
            tc.tile_pool(name="psA", bufs=1, space="PSUM") as psA,
            tc.tile_pool(name="psB", bufs=2, space="PSUM") as psB,
            tc.tile_pool(name="dram", bufs=1, space="DRAM") as dpool,
        ):
            # identity needed by the very first transposes
            eye_sb = wpool.tile([128, 128], fp32)
            nc.scalar.dma_start(out=eye_sb, in_=eye128.ap())

            # leftover node stream (nodes 128..134), all batches, early;
            # host pre-transposed to [128, B*28] so this is one clean DMA
            llt = lpool.tile([128, B, 28], fp32)
            nc.scalar.dma_start(
                out=llt, in_=xsl.ap().rearrange("p (b f) -> p b f", b=B))

            # main group DMAs issued up-front in program order.
            # Batch reduction on TensorE: psum += I.T @ tile_b, identity
            # stationary loaded once; X_bar lands node-major in one bank.
            PEW = 384                            # PE cols; DVE does the rest
            px = psA.tile([128, PEW], fp32)      # one psum bank
            acc3 = apool.tile([128, F - PEW], fp32)
            for g in range(NGROUPS):
                gtm = spool.tile([128, GB, F], fp32, tag="grp")
                src = xsm.ap()[g * GB:(g + 1) * GB, :].rearrange(
                    "b (n f) -> n b f", n=128)
                nc.sync.dma_start(out=gtm, in_=src)
                for b in range(GB):
                    bg = g * GB + b
                    nc.tensor.matmul(px, eye_sb, gtm[:, b, 0:PEW],
                                     start=(bg == 0), stop=(bg == B - 1))
                    if bg == 0:
                        nc.vector.tensor_copy(out=acc3, in_=gtm[:, 0, PEW:F])
                    else:
                        nc.vector.tensor_add(out=acc3, in0=acc3,
                                             in1=gtm[:, b, PEW:F])

            # ---- weights (scheduled around the stream) ----
            w1pat_sb = wpool.tile([128, F], fp32)
            nc.scalar.dma_start(out=w1pat_sb, in_=w1pat.ap())
            w1patl_sb = wpool.tile([7, F], fp32)
            nc.scalar.dma_start(out=w1patl_sb, in_=w1patl.ap())
            sel_sb = wpool.tile([128, NW], fp32)
            nc.scalar.dma_start(out=sel_sb, in_=sel.ap())
            sel2_sb = wpool.tile([7, NW], fp32)
            nc.scalar.dma_start(out=sel2_sb, in_=sel2.ap())
            p1w_sb = wpool.tile([128, 4, H1], fp32)
            nc.scalar.dma_start(out=p1w_sb, in_=p1wr.ap())
            b1_sb = wpool.tile([112, 7], fp32)
            nc.scalar.dma_start(out=b1_sb, in_=b1r.ap())
            nc2w_sb = wpool.tile([112, 7, P], fp32)
            nc.scalar.dma_start(out=nc2w_sb, in_=nc2wr.ap())
            p2w_sb = wpool.tile([112, 7, H2], fp32)
            nc.scalar.dma_start(out=p2w_sb, in_=p2wr.ap())
            b2_sb = wpool.tile([H2, 1], fp32)
            nc.scalar.dma_start(out=b2_sb, in_=b2c.ap())
            cw1_sb = wpool.tile([H2, P, 32], fp32)
            nc.scalar.dma_start(out=cw1_sb, in_=cw1.ap())
            cb1_sb = wpool.tile([32, 1], fp32)
            nc.scalar.dma_start(out=cb1_sb, in_=cb1c.ap())
            cw2_sb = wpool.tile([32, NCLS], fp32)
            nc.scalar.dma_start(out=cw2_sb, in_=cw2.ap())
            cb2_sb = wpool.tile([NCLS, 1], fp32)
            nc.scalar.dma_start(out=cb2_sb, in_=cb2c.ap())

            # preload the gelu ACT table during phase A
            gdummy = tpool.tile([H2, 1], fp32)
            nc.scalar.activation(out=gdummy, in_=b2_sb, func=Gelu)

            # leftover reduction: 63 adds of [128, 28] + roundtrip
            # (both hidden under phase A)
            accl = apool.tile([128, 28], fp32)
            for b in range(B):
                if b == 0:
                    nc.vector.tensor_copy(out=accl, in_=llt[:, 0, :])
                else:
                    nc.vector.tensor_add(out=accl, in0=accl, in1=llt[:, b, :])
            scratch = dpool.tile([LEFT_ELEMS], fp32)
            nc.sync.dma_start(
                out=scratch.rearrange("(p f) -> p f", p=128), in_=accl)
            lt7 = lpool.tile([7, F], fp32)
            nc.sync.dma_start(
                out=lt7, in_=scratch.rearrange("(n f) -> n f", n=7))
            yl = lpool.tile([7, F], fp32)
            nc.vector.tensor_mul(out=yl, in0=lt7, in1=w1patl_sb)

            # ---- after the stream: drain X_bar, apply window weights ----
            xbar = tpool.tile([128, F], fp32)
            nc.vector.tensor_copy(out=xbar[:, 0:PEW], in_=px)
            nc.vector.tensor_copy(out=xbar[:, PEW:F], in_=acc3)
            ymain = tpool.tile([128, F], fp32)
            nc.vector.tensor_mul(out=ymain, in0=xbar, in1=w1pat_sb)

            # hsumT[f, s] = sum_{n in window s} Y[n, f]   (Y^T @ Sel on PE)
            hsumT = tpool.tile([128, 4, NW], fp32)
            for fc in range(4):
                ph = psB.tile([128, NW], fp32, tag="ph")
                nc.tensor.matmul(ph, ymain[:, fc * 128:(fc + 1) * 128],
                                 sel_sb, start=True, stop=False)
                nc.tensor.matmul(ph, yl[:, fc * 128:(fc + 1) * 128],
                                 sel2_sb, start=False, stop=True)
                nc.vector.tensor_copy(out=hsumT[:, fc, :], in_=ph)

            # ---- M1^T chunks + gelu -> h1cT [112, 7, 45] ----
            h1cT = tpool.tile([112, 7, NW], fp32)
            for hc in range(7):
                pm = psB.tile([112, NW], fp32, tag="pm")
                for fc in range(4):
                    lhsT = p1w_sb[:, fc, hc * 112:(hc + 1) * 112]
                    nc.tensor.matmul(pm, lhsT, hsumT[:, fc, :],
                                     start=(fc == 0), stop=(fc == 3))
                nc.scalar.activation(out=h1cT[:, hc, :], in_=pm, func=Gelu,
                                     bias=b1_sb[:, hc:hc + 1], scale=360.0)

            # ---- node conv 2 (64x batch factor folded into nc2w host-side) ----
            tmp2 = tpool.tile([112, 7, NW], fp32)
            h1v = h1cT.rearrange("p c (s q) -> p c s q", q=P)
            w2v = nc2w_sb[:, :, None, :].to_broadcast((112, 7, S2, P))
            nc.vector.tensor_mul(
                out=tmp2.rearrange("p c (s q) -> p c s q", q=P),
                in0=h1v, in1=w2v)
            hs2T = tpool.tile([112, 7, S2], fp32)
            nc.vector.reduce_sum(
                out=hs2T, in_=tmp2.rearrange("p c (s q) -> p c s q", q=P),
                axis=mybir.AxisListType.X)

            # ---- M2^T [28, 15] + gelu ----
            pm2 = psB.tile([H2, S2], fp32, tag="pm")
            for c in range(7):
                nc.tensor.matmul(pm2, p2w_sb[:, c, :], hs2T[:, c, :],
                                 start=(c == 0), stop=(c == 6))
            out2T = tpool.tile([H2, S2], fp32)
            nc.scalar.activation(out=out2T, in_=pm2, func=Gelu,
                                 bias=b2_sb[:, 0:1], scale=120.0)

            # ---- classifier ----
            o2v = out2T.rearrange("h (r q) -> h r q", q=P)
            pc1 = psB.tile([32, CR], fp32, tag="pm")
            for q in range(P):
                nc.tensor.matmul(pc1, cw1_sb[:, q, :], o2v[:, :, q],
                                 start=(q == 0), stop=(q == P - 1))
            c1T = tpool.tile([32, CR], fp32)
            nc.scalar.activation(out=c1T, in_=pc1, func=Gelu,
                                 bias=cb1_sb[:, 0:1], scale=1.0)
            pc2 = psB.tile([NCLS, CR], fp32, tag="pm")
            nc.tensor.matmul(pc2, cw2_sb, c1T, start=True, stop=True)
            outT = tpool.tile([NCLS, CR], fp32)
            nc.scalar.activation(out=outT, in_=pc2, func=Ident,
                                 bias=cb2_sb[:, 0:1], scale=1.0)
            nc.sync.dma_start(out=out.ap(), in_=outT)

    nc.compile()
    return nc


def _prep_in_maps(inputs):
    x = np.ascontiguousarray(np.asarray(inputs["x"], dtype=np.float32))
    nc1_w = np.asarray(inputs["nc1_w"], dtype=np.float32)
    prop1_W = np.asarray(inputs["prop1_W"], dtype=np.float32)
    prop1_b = np.asarray(inputs["prop1_b"], dtype=np.float32)
    nc2_w = np.asarray(inputs["nc2_w"], dtype=np.float32)
    prop2_W = np.asarray(inputs["prop2_W"], dtype=np.float32)
    prop2_b = np.asarray(inputs["prop2_b"], dtype=np.float32)
    cls_w1 = np.asarray(inputs["cls_w1"], dtype=np.float32)
    cls_b1 = np.asarray(inputs["cls_b1"], dtype=np.float32)
    cls_w2 = np.asarray(inputs["cls_w2"], dtype=np.float32)
    cls_b2 = np.asarray(inputs["cls_b2"], dtype=np.float32)

    common = {
        "w1pat": np.ascontiguousarray(
            nc1_w[np.arange(128) % P, :]),
        "w1patl": np.ascontiguousarray(
            nc1_w[(128 + np.arange(7)) % P, :]),
        "sel": np.ascontiguousarray(
            (np.arange(128)[:, None] // P == np.arange(NW)[None, :])
            .astype(np.float32)),
        "sel2": np.ascontiguousarray(
            ((128 + np.arange(7))[:, None] // P == np.arange(NW)[None, :])
            .astype(np.float32)),
        "eye128": np.eye(128, dtype=np.float32),
        "p1wr": np.ascontiguousarray(
            prop1_W.reshape(4, 128, H1).swapaxes(0, 1)),
        "b1r": np.ascontiguousarray(prop1_b.reshape(7, 112).T),
        "nc2wr": np.ascontiguousarray(
            (64.0 * nc2_w).astype(np.float32).T.reshape(7, 112, P)
            .swapaxes(0, 1)),
        "p2wr": np.ascontiguousarray(prop2_W.reshape(7, 112, H2)
                                     .swapaxes(0, 1)),
        "b2c": np.ascontiguousarray(prop2_b.reshape(H2, 1)),
        "cw1": np.ascontiguousarray(cls_w1.reshape(P, H2, 32).swapaxes(0, 1)),
        "cb1c": np.ascontiguousarray(cls_b1.reshape(32, 1)),
        "cw2": np.ascontiguousarray(cls_w2),
        "cb2c": np.ascontiguousarray(cls_b2.reshape(NCLS, 1)),
    }
    in_maps = []
    for c in range(NCORES):
        xsc = x[:, c * SLICE_N:(c + 1) * SLICE_N, :].reshape(B, SLICE_ELEMS)
        xsm = np.ascontiguousarray(xsc[:, :MAIN_ELEMS])
        xsl = np.ascontiguousarray(
            xsc[:, MAIN_ELEMS:].reshape(B, 128, 28).transpose(1, 0, 2)
            .reshape(128, B * 28))
        in_maps.append({"xsm": xsm, "xsl": xsl, **common})
    return in_maps


def run(inputs, trace=False):
    from concourse import bass_utils
    if "nc" not in _CACHE:
        _CACHE["nc"] = _build_bass()
    nc = _CACHE["nc"]
    in_maps = _prep_in_maps(inputs)
    res = bass_utils.run_bass_kernel_spmd(
        nc, in_maps, core_ids=list(range(NCORES)), trace=trace)
    outs = [np.asarray(res.results[c]["out"]) for c in range(NCORES)]
    block = np.concatenate([o.T for o in outs], axis=0)       # [40, 10]
    full = np.tile(block, (B, 1)).astype(np.float32)          # [2560, 10]
    return full, res


def kernel(**inputs) -> np.ndarray:
    out, _ = run(inputs, trace=False)
    return out
